# revision 1
# baseline (speedup 1.0000x reference)
"""Trainium2 Bass kernel for nn_GateCircuit (14-qubit batched gate circuit).

Math: the reference applies RX(x@W.T[:,i]) then RY(params[i]) on wire i of
|0...0> (a product state stays a product state since each gate hits a distinct
wire), then a CNOT ladder CNOT(i, i+1), then measures <Z_0>.  Qubit 0 is only
ever a CNOT *control*, so its marginal is untouched by the ladder; the
expectation collapses to the single-qubit value

    <Z_0> = cos(x @ W[0]) * cos(params[0])
    out   = sigmoid(<Z_0>)

Sharding: pure data parallel, batch 4096 split 512 per core across 8 cores;
W row 0 and params[0] replicated (pre-broadcast host-side, no host math).

On-device per core (all f32):
  z_b = sum_f x[b,f] * W0[f]        4x scalar_tensor_tensor + DVE accumulator
  range-reduce y = z - 2pi*k        k = float(int(z/2pi)); |y| < 2pi either
                                    rounding convention; only sin^2(y/2) is
                                    used so the k convention cannot matter
  s = Sin(y*0.5); q = Square(s)     ACT (one table set: trig_and_small)
  a = q*(-2c0) + c0 = c0*cos(z)     c0 = cos(params[0]) from the same
                                    range-reduce+sin pipeline on GpSimd/ACT
  u = Square(q*(-2c0) + c0) = a^2   ACT, scale/bias are per-partition APs
  sigmoid(a) = 0.5 + a*g(u)         odd minimax poly, deg 3 in u, err 3e-7
DMAs are spread across both HWDGE rings (SP via nc.sync, ACT via nc.scalar)
so descriptor injection and the ~2us completion latencies pipeline.
"""

import math

import numpy as np

_NCORES = 8
_B = 4096
_F = 256
_BS = _B // _NCORES  # 512 samples per core
_NT = _BS // 128     # 4 sample-tiles of 128 partitions
_TWO_PI = float(2.0 * math.pi)
_INV_TWO_PI = float(1.0 / (2.0 * math.pi))
# sigmoid(a) - 0.5 = a * (C0 + C1*u + C2*u^2 + C3*u^3), u = a^2, a in [-1,1]
_C0 = 0.249999905
_C1 = -0.0208298861
_C2 = 0.00206395637
_C3 = -0.000175724981

_CACHE: dict = {}


def _build():
    import concourse.bacc as bacc
    import concourse.mybir as mybir
    import concourse.tile as tile

    f32 = mybir.dt.float32
    i32 = mybir.dt.int32
    Alu = mybir.AluOpType
    Act = mybir.ActivationFunctionType

    nc = bacc.Bacc("TRN2", target_bir_lowering=False, debug=False,
                   num_devices=_NCORES)

    x_d = nc.dram_tensor("x", [_BS, _F], f32, kind="ExternalInput")
    w_d = nc.dram_tensor("w", [128, _F], f32, kind="ExternalInput")   # W[0] bcast
    p_d = nc.dram_tensor("p", [128, 1], f32, kind="ExternalInput")    # params[0] bcast
    o_d = nc.dram_tensor("o", [_BS], f32, kind="ExternalOutput")

    with tile.TileContext(nc) as tc:
        with (
            tc.tile_pool(name="const", bufs=1) as cpool,
            tc.tile_pool(name="xin", bufs=1) as xpool,
            tc.tile_pool(name="scratch", bufs=2) as spool,
            tc.tile_pool(name="small", bufs=1) as zpool,
        ):
            # --- input DMAs: alternate the two HWDGE rings (SP / ACT);
            # p first so its constants are ready long before the tail ---
            p_b = cpool.tile([128, 1], f32)
            nc.sync.dma_start(p_b[:], p_d[:, :])
            w_b = cpool.tile([128, _F], f32)
            nc.scalar.dma_start(w_b[:], w_d[:, :])
            xr = x_d.ap().rearrange("(p n) f -> n p f", n=_NT)
            xts = []
            for n in range(_NT):
                xt = xpool.tile([128, _F], f32, name=f"xt{n}")
                eng = nc.sync if n % 2 == 0 else nc.scalar
                eng.dma_start(xt[:], xr[n])
                xts.append(xt)

            # --- dot products z[:, n] = sum_f x_tile_n * w  (DVE) ---
            z = zpool.tile([128, _NT], f32)
            for n in range(_NT):
                prod = spool.tile([128, _F], f32)
                nc.vector.scalar_tensor_tensor(
                    prod[:], xts[n][:], 1.0, w_b[:],
                    op0=Alu.mult, op1=Alu.mult,
                    accum_out=z[:, n:n + 1],
                )

            # --- cos(params[0]) constants (GpSimd + ACT, off DVE path) ---
            tp = zpool.tile([128, 1], f32)
            nc.gpsimd.tensor_scalar_mul(tp[:], p_b[:], _INV_TWO_PI)
            kpi = zpool.tile([128, 1], i32)
            nc.gpsimd.tensor_copy(kpi[:], tp[:])
            kpf = zpool.tile([128, 1], f32)
            nc.gpsimd.tensor_copy(kpf[:], kpi[:])
            kps = zpool.tile([128, 1], f32)
            nc.gpsimd.tensor_scalar_mul(kps[:], kpf[:], -_TWO_PI)
            yp = zpool.tile([128, 1], f32)
            nc.gpsimd.tensor_tensor(yp[:], kps[:], p_b[:], op=Alu.add)
            sp_t = zpool.tile([128, 1], f32)
            nc.scalar.activation(sp_t[:], yp[:], Act.Sin, scale=0.5)
            q0 = zpool.tile([128, 1], f32)   # sin^2(p0/2)
            nc.gpsimd.tensor_tensor(q0[:], sp_t[:], sp_t[:], op=Alu.mult)
            # negc2 = 4*q0 - 2 = -2*cos(p0);  cpos = 1 - 2*q0 = cos(p0)
            negc2 = zpool.tile([128, 1], f32)
            nc.gpsimd.tensor_scalar(negc2[:], q0[:], 4.0, -2.0,
                                    op0=Alu.mult, op1=Alu.add)
            cpos = zpool.tile([128, 1], f32)
            nc.gpsimd.tensor_scalar(cpos[:], q0[:], -2.0, 1.0,
                                    op0=Alu.mult, op1=Alu.add)

            # --- range reduce z, sin^2(z/2) ---
            tz = zpool.tile([128, _NT], f32)
            nc.vector.tensor_scalar_mul(tz[:], z[:], _INV_TWO_PI)
            kzi = zpool.tile([128, _NT], i32)
            nc.vector.tensor_copy(kzi[:], tz[:])
            kzf = zpool.tile([128, _NT], f32)
            nc.vector.tensor_copy(kzf[:], kzi[:])
            yz = zpool.tile([128, _NT], f32)
            nc.vector.scalar_tensor_tensor(yz[:], kzf[:], -_TWO_PI, z[:],
                                           op0=Alu.mult, op1=Alu.add)
            sz = zpool.tile([128, _NT], f32)
            nc.scalar.activation(sz[:], yz[:], Act.Sin, scale=0.5)
            q = zpool.tile([128, _NT], f32)  # sin^2(z/2)
            nc.vector.tensor_tensor(q[:], sz[:], sz[:], op=Alu.mult)
            a = zpool.tile([128, _NT], f32)  # a = c0*cos(z)
            nc.vector.tensor_scalar(a[:], q[:], negc2[:, :], cpos[:, :],
                                    op0=Alu.mult, op1=Alu.add)
            u = zpool.tile([128, _NT], f32)  # u = a^2
            nc.vector.tensor_tensor(u[:], a[:], a[:], op=Alu.mult)

            # --- sigmoid via odd poly: out = 0.5 + a*(C0 + C1 u + C2 u^2 + C3 u^3)
            q1 = zpool.tile([128, _NT], f32)
            nc.vector.tensor_scalar(q1[:], u[:], _C3, _C2,
                                    op0=Alu.mult, op1=Alu.add)
            q2 = zpool.tile([128, _NT], f32)
            nc.vector.scalar_tensor_tensor(q2[:], q1[:], 0.0, u[:],
                                           op0=Alu.bypass, op1=Alu.mult)
            g1 = zpool.tile([128, _NT], f32)
            nc.vector.scalar_tensor_tensor(g1[:], q2[:], _C1, u[:],
                                           op0=Alu.add, op1=Alu.mult)
            o2 = zpool.tile([128, _NT], f32)
            nc.vector.scalar_tensor_tensor(o2[:], g1[:], _C0, a[:],
                                           op0=Alu.add, op1=Alu.mult)
            ot = zpool.tile([128, _NT], f32)
            nc.vector.tensor_scalar(ot[:], o2[:], 1.0, 0.5,
                                    op0=Alu.mult, op1=Alu.add)

            nc.sync.dma_start(o_d.ap().rearrange("(p n) -> p n", n=_NT), ot[:])

    nc.compile()
    return nc


def _get_nc():
    if "nc" not in _CACHE:
        _CACHE["nc"] = _build()
    return _CACHE["nc"]


def _in_maps(x, W, params):
    x = np.ascontiguousarray(np.asarray(x, dtype=np.float32))
    W = np.asarray(W, dtype=np.float32)
    params = np.asarray(params, dtype=np.float32)
    w_b = np.ascontiguousarray(np.broadcast_to(W[0:1, :], (128, _F)))
    p_b = np.full((128, 1), params[0], dtype=np.float32)
    return [
        {"x": x[c * _BS:(c + 1) * _BS], "w": w_b, "p": p_b}
        for c in range(_NCORES)
    ]


def run_spmd(x, W, params, **kw):
    """Compile (cached) and run on 8 cores; returns BassKernelResults.

    Retries a few times: the axon-relayed device occasionally reports a
    transient NRT_EXEC_UNIT_UNRECOVERABLE that clears on the next attempt.
    """
    import time

    from concourse import bass_utils

    nc = _get_nc()
    in_maps = _in_maps(x, W, params)
    last = None
    for attempt in range(4):
        try:
            return bass_utils.run_bass_kernel_spmd(
                nc, in_maps, list(range(_NCORES)), **kw
            )
        except Exception as e:  # transient device/relay errors
            last = e
            time.sleep(2.0 * (attempt + 1))
    raise last


def kernel(x, W, params):
    res = run_spmd(x, W, params)
    return np.concatenate([res.results[c]["o"] for c in range(_NCORES)], axis=0)

